# revision 36
# baseline (speedup 1.0000x reference)
"""Trainium2 Bass kernel for nn_FP8Experts (MoE with FP8 block-quantized experts).

Strategy (expert-parallel over 8 NeuronCores):
  - Host: route tokens to experts by top_k_index (each expert's token list,
    padded to a common capacity C); fully dequantize the fp8 block-quantized
    weights to fp16 (w = q * block_scale); apply the reference's dynamic
    per-token/per-128-block fp8 act-quant round-trip to the routed
    activations (bit-exact reference semantics, rounded once to fp16); lay
    both out contraction-major + chunk-major so every DMA is one fat
    contiguous per-partition run.
  - Device (per core = one expert): fp16 weights resident in SBUF, fp16
    matmuls (gate_up -> silu*up -> act-quant of the intermediate -> down)
    accumulated in PSUM fp32. The intermediate act-quant (per-token,
    per-128-block fp8 round-trip matching the reference on a /2-scaled grid:
    224 = 448/2, exact vs OCP e4m3fn away from the denormal floor) runs on
    the vector/scalar engines, overlapped with the matmul stream; its
    transposes (contraction-major for the PE) run on the tensor engine, each
    delayed one chunk-slot so the quant chain's latency stays hidden.
    The first two token tiles are processed chunk-interleaved so the PE
    trails the weight-DMA wavefront without stalling.
  - Host: weighted combine with top_k_weights.
"""

import numpy as np
import ml_dtypes

E, H, I = 8, 2048, 1408
T, TOPK = 4096, 2
BN = BK = 128
NCORES = 8
P = 128
HALF_MAX = 224.0

F8_OCP = ml_dtypes.float8_e4m3fn   # reference grid (max 448)

# gate/up column chunks in matmul-consumption order (g0,u0,g1,u1,g2,u2):
# (orig column offset in [0, 2816), width)
GU_CH = [(0, 512), (1408, 512), (512, 512), (1920, 512),
         (1024, 384), (2432, 384)]
GU_BASE = np.cumsum([0] + [cw for _, cw in GU_CH]).tolist()  # flat offsets
GU_TOT = GU_BASE[-1]            # 2816
KB1 = H // BK                   # 16 contraction blocks for gate_up
KB2 = I // BK                   # 11 contraction blocks for down
WD_CW = 512
WD_TOT = KB2 * H                # flat down-weight cols (chunk-major)

# gate/up paired column chunks: (offset-within-half, width, #inter-blocks)
GCHUNKS = [(0, 512, 4), (512, 512, 4), (1024, 384, 3)]

_compiled_cache = {}
_weights_cache = {}


def _build(C):
    """Build + schedule the per-core Bass kernel for token capacity C."""
    import concourse.bass as bass
    import concourse.mybir as mybir
    import concourse.tile as tile
    from concourse import bacc

    f32 = mybir.dt.float32
    f16 = mybir.dt.float16
    f8 = mybir.dt.float8e4
    AF = mybir.ActivationFunctionType
    ALU = mybir.AluOpType
    AX = mybir.AxisListType

    NT = C // P

    nc = bacc.Bacc("TRN2", target_bir_lowering=False, debug=False,
                   num_devices=NCORES)

    xq_d = nc.dram_tensor("xqt", [NT, P, H], f16, kind="ExternalInput").ap()
    wgu_d = nc.dram_tensor("wgu16", [P, GU_TOT * KB1], f16,
                           kind="ExternalInput").ap()
    wd_d = nc.dram_tensor("wd16", [P, WD_TOT], f16, kind="ExternalInput").ap()
    id_d = nc.dram_tensor("ident", [P, P], f16, kind="ExternalInput").ap()
    y_d = nc.dram_tensor("y", [C, H], f32, kind="ExternalOutput").ap()

    with tile.TileContext(nc) as tc:
        with (
            tc.tile_pool(name="const", bufs=1) as const,
            tc.tile_pool(name="wpool", bufs=1) as wpool,
            tc.tile_pool(name="qp", bufs=2) as qp,
            tc.tile_pool(name="tp", bufs=2) as tp,
            tc.tile_pool(name="pp", bufs=6, space="PSUM") as pp,
            tc.tile_pool(name="pt", bufs=2, space="PSUM") as pt,
        ):
            ident = const.tile([P, P], f16, name="ident")
            nc.sync.dma_start(ident[:], id_d[:])

            # first use of each engine opcode pays a cold uop-table load;
            # warm every opcode the pipeline uses on tiny tiles first
            wu8 = const.tile([P, 8], f8, name="wu8")
            wu16 = const.tile([P, 8], f16, name="wu16")
            wu32 = const.tile([P, 8], f32, name="wu32")
            nc.vector.tensor_copy(out=wu32[:], in_=ident[:, :8])
            nc.vector.reduce_max(wu32[:, :1], wu32[:, :8], axis=AX.X,
                                 apply_absolute_value=True)
            nc.vector.tensor_scalar_max(wu32[:], wu32[:], 1e-12)
            nc.vector.reciprocal(wu32[:], wu32[:])
            nc.vector.tensor_scalar_mul(wu32[:], wu32[:], 1.0)
            nc.vector.tensor_tensor(out=wu8[:], in0=wu32[:], in1=wu32[:],
                                    op=ALU.mult)
            nc.vector.tensor_tensor(out=wu16[:], in0=wu8[:], in1=wu32[:],
                                    op=ALU.mult)
            nc.scalar.activation(wu16[:], wu16[:], AF.Silu)
            nc.scalar.activation(wu16[:], wu16[:], AF.Copy, scale=1.0)

            # PE warmup: a few dummy matmuls bridge until the first weight
            # quarter-chunk + tile-0 activations land (real matmuls then
            # finish waking the HAM clock-gate).
            ps_warm = pp.tile([P, 512], f32, name="ps", tag="ps")
            for _ in range(44):
                nc.tensor.matmul(ps_warm[:, :P], lhsT=ident[:], rhs=ident[:],
                                 start=True, stop=True)

            # ---------------- resident fp16 weights (host-dequantized) -----
            # chunk-major flat layouts: fat contiguous per-partition DMA runs
            wgu_all = wpool.tile([P, GU_TOT * KB1], f16, name="wgu_all")
            wd_all = wpool.tile([P, WD_TOT], f16, name="wd_all")

            def gu_rhs(ci, kb):
                cw = GU_CH[ci][1]
                b = GU_BASE[ci] * KB1 + kb * cw
                return wgu_all[:, b:b + cw]

            def wd_rhs(hc, kb):
                b = hc * (KB2 * WD_CW) + kb * WD_CW
                return wd_all[:, b:b + WD_CW]

            def load_xq(tt):
                xqT = qp.tile([P, H], f16, name="xqT", tag="xqT")
                nc.sync.dma_start(xqT[:], xq_d[tt])
                return xqT

            # tile 0/1 activation tiles (DMAs issued below, interleaved with
            # the weight stream in first-consumption order)
            hoisted = {0: qp.tile([P, H], f16, name="xqT", tag="xqT")}
            if NT > 1:
                hoisted[1] = qp.tile([P, H], f16, name="xqT", tag="xqT")

            def wdma(lo, hi):
                nc.sync.dma_start(wgu_all[:, lo:hi], wgu_d[:, lo:hi])

            # issue order = order of first consumption by the PE: quarter
            # chunks of the first gate/up pair interleaved with tile-0/1
            # activations, then the remaining chunks in halves
            Q = 4 * 512                                  # kb-quarter cols
            wdma(0, Q)                                   # g0 kb0-3
            nc.sync.dma_start(hoisted[0][:, :H // 2], xq_d[0][:, :H // 2])
            nc.sync.dma_start(hoisted[0][:, H // 2:], xq_d[0][:, H // 2:])
            wdma(Q, 2 * Q)                               # g0 kb4-7
            wdma(2 * Q, 3 * Q)                           # g0 kb8-11
            wdma(3 * Q, 4 * Q)                           # g0 kb12-15
            wdma(4 * Q, 5 * Q)                           # u0 kb0-3
            wdma(5 * Q, 6 * Q)                           # u0 kb4-7
            if NT > 1:
                nc.sync.dma_start(hoisted[1][:], xq_d[1])
            wdma(6 * Q, 7 * Q)                           # u0 kb8-11
            wdma(7 * Q, 8 * Q)                           # u0 kb12-15
            for ci in range(2, len(GU_CH)):
                cw = GU_CH[ci][1]
                b = GU_BASE[ci] * KB1
                half = (KB1 // 2) * cw
                wdma(b, b + half)
                wdma(b + half, b + KB1 * cw)
            for hc in range(4):
                b = hc * (KB2 * WD_CW)
                half = 6 * WD_CW
                nc.sync.dma_start(wd_all[:, b:b + half], wd_d[:, b:b + half])
                nc.sync.dma_start(wd_all[:, b + half:b + KB2 * WD_CW],
                                  wd_d[:, b + half:b + KB2 * WD_CW])

            # ---------------- per-tile emission helpers --------------------
            def tile_begin(tt):
                xqT = hoisted.pop(tt, None)
                if xqT is None:
                    xqT = load_xq(tt)
                if tt + 1 < NT and tt + 1 not in hoisted:
                    hoisted[tt + 1] = load_xq(tt + 1)
                return {
                    "xqT": xqT,
                    "iq16": qp.tile([P, KB2, BK], f16, name="iq16", tag="iq16"),
                    "iqT": qp.tile([P, KB2, BK], f16, name="iqT", tag="iqT"),
                    "amax": qp.tile([P, KB2], f32, name="amax_i", tag="amax_i"),
                    "inv": qp.tile([P, KB2], f32, name="inv_i", tag="inv_i"),
                    "s2": qp.tile([P, KB2], f32, name="s2_i", tag="s2_i"),
                }

            def emit_pair(st, gi):
                """gate+up matmuls for chunk gi, then silu*up + act-quant of
                the resulting intermediate blocks (vector/scalar engines)."""
                off, w, nb = GCHUNKS[gi]
                xqT = st["xqT"]
                ps_g = pp.tile([P, 512], f32, name="ps", tag="ps")[:, :w]
                for kb in range(KB1):
                    nc.tensor.matmul(ps_g, lhsT=xqT[:, kb * BK:(kb + 1) * BK],
                                     rhs=gu_rhs(2 * gi, kb),
                                     start=(kb == 0), stop=(kb == KB1 - 1))
                ps_u = pp.tile([P, 512], f32, name="ps", tag="ps")[:, :w]
                for kb in range(KB1):
                    nc.tensor.matmul(ps_u, lhsT=xqT[:, kb * BK:(kb + 1) * BK],
                                     rhs=gu_rhs(2 * gi + 1, kb),
                                     start=(kb == 0), stop=(kb == KB1 - 1))
                sil = tp.tile([P, 512], f32, name="sil", tag="sil")[:, :w]
                nc.scalar.activation(sil, ps_g, AF.Silu)
                itc = tp.tile([P, 512], f32, name="itc", tag="itc")[:, :w]
                nc.vector.tensor_mul(itc, sil, ps_u)

                b0 = off // BN
                am = st["amax"][:, b0:b0 + nb]
                nc.vector.reduce_max(
                    am, itc.rearrange("p (b k) -> p b k", k=BK),
                    axis=AX.X, apply_absolute_value=True,
                )
                nc.vector.tensor_scalar_max(am, am, 1e-12)
                nc.vector.reciprocal(st["inv"][:, b0:b0 + nb], am)
                nc.vector.tensor_scalar_mul(st["inv"][:, b0:b0 + nb],
                                            st["inv"][:, b0:b0 + nb], HALF_MAX)
                nc.vector.tensor_scalar_mul(st["s2"][:, b0:b0 + nb], am,
                                            1.0 / HALF_MAX)
                qi8 = tp.tile([P, 512], f8, name="qi8", tag="qi8")[:, :w]
                nc.vector.tensor_tensor(
                    out=qi8.rearrange("p (b k) -> p b k", k=BK),
                    in0=itc.rearrange("p (b k) -> p b k", k=BK),
                    in1=st["inv"][:, b0:b0 + nb, None].to_broadcast(
                        [P, nb, BK]),
                    op=ALU.mult,
                )
                # fp8-input DVE ops are slow; split the dequant-to-fp16
                # between DVE and ACT
                nd = nb // 2
                nc.vector.tensor_tensor(
                    out=st["iq16"][:, b0:b0 + nd, :],
                    in0=qi8.rearrange("p (b k) -> p b k", k=BK)[:, :nd],
                    in1=st["s2"][:, b0:b0 + nd, None].to_broadcast(
                        [P, nd, BK]),
                    op=ALU.mult,
                )
                for b in range(nd, nb):
                    nc.scalar.activation(
                        st["iq16"][:, b0 + b, :], qi8[:, b * BK:(b + 1) * BK],
                        AF.Copy, scale=st["s2"][:, b0 + b:b0 + b + 1])

            def emit_T(st, gi):
                """PE transpose of chunk gi's quantized intermediate blocks:
                [token, feat] -> [feat, token]."""
                off, w, nb = GCHUNKS[gi]
                b0 = off // BN
                ps_t = pt.tile([P, 4, P], f16, name="ps_t", tag="ps_t")
                for j in range(nb):
                    nc.tensor.transpose(ps_t[:, j, :],
                                        st["iq16"][:, b0 + j, :], ident[:])
                nc.vector.tensor_copy(out=st["iqT"][:, b0:b0 + nb, :],
                                      in_=ps_t[:, :nb, :])

            def emit_down(st, tt):
                for hc in range(4):
                    ps_y = pp.tile([P, 512], f32, name="ps", tag="ps")
                    for kb in range(KB2):
                        nc.tensor.matmul(ps_y, lhsT=st["iqT"][:, kb, :],
                                         rhs=wd_rhs(hc, kb),
                                         start=(kb == 0), stop=(kb == KB2 - 1))
                    yt = tp.tile([P, 512], f32, name="yt", tag="yt")
                    nc.scalar.copy(yt[:], ps_y[:])
                    nc.sync.dma_start(
                        y_d[tt * P:(tt + 1) * P, hc * 512:(hc + 1) * 512],
                        yt[:])

            # ---------------- main loop over 128-token tiles ----------------
            # Tiles 0/1 run chunk-interleaved: per chunk the PE has ~13.6us
            # of matmuls against ~6us of weight DMA, so it never stalls on
            # the weight stream. Transposes trail by one chunk slot.
            start = 0
            if NT >= 2:
                stA, stB = tile_begin(0), tile_begin(1)
                for gi in range(3):
                    for st in (stA, stB):
                        if gi > 0:
                            emit_T(st, gi - 1)
                        emit_pair(st, gi)
                emit_T(stA, 2)
                emit_down(stA, 0)
                emit_T(stB, 2)
                emit_down(stB, 1)
                start = 2
            for tt in range(start, NT):
                st = tile_begin(tt)
                emit_pair(st, 0)
                emit_pair(st, 1)
                emit_T(st, 0)
                emit_pair(st, 2)
                emit_T(st, 1)
                emit_T(st, 2)
                emit_down(st, tt)

    nc.compile()
    return nc


def _prep_weights(gate_up_proj, gate_up_proj_scale_inv, down_proj,
                  down_proj_scale_inv):
    """Per-expert fully dequantized fp16 weights (w = q * block_scale), in
    chunk-major contraction-major flat layout for fat contiguous DMAs."""
    cached = _weights_cache.get("w")
    if cached is not None and cached[0] is gate_up_proj \
            and cached[1] is down_proj:
        return cached[2]
    NB1, NB2 = 2 * I // BN, H // BN
    out = []
    gup = np.asarray(gate_up_proj)
    gus = np.asarray(gate_up_proj_scale_inv, dtype=np.float32)
    dwn = np.asarray(down_proj)
    dws = np.asarray(down_proj_scale_inv, dtype=np.float32)
    for e in range(E):
        w32 = gup[e].astype(np.float32).reshape(NB1, BN, KB1, BK)
        w32 *= gus[e][:, None, :, None]
        w16T = w32.reshape(2 * I, H).T.astype(np.float16)   # [H, 2I]
        parts = []
        for o, cw in GU_CH:
            blk = w16T[:, o:o + cw].reshape(KB1, P, cw)
            parts.append(blk.transpose(1, 0, 2).reshape(P, KB1 * cw))
        wgu = np.ascontiguousarray(np.concatenate(parts, axis=1))
        w32 = dwn[e].astype(np.float32).reshape(NB2, BN, KB2, BK)
        w32 *= dws[e][:, None, :, None]
        wdT = w32.reshape(H, I).T.astype(np.float16)        # [I, H]
        parts = []
        for hc in range(4):
            blk = wdT[:, hc * WD_CW:(hc + 1) * WD_CW].reshape(KB2, P, WD_CW)
            parts.append(blk.transpose(1, 0, 2).reshape(P, KB2 * WD_CW))
        wd = np.ascontiguousarray(np.concatenate(parts, axis=1))
        out.append((wgu, wd))
    _weights_cache["w"] = (gate_up_proj, down_proj, out)
    return out


def _act_quant_fp16(x):
    """Reference _act_quant_dequant (per-token, per-128-block OCP e4m3fn
    round-trip), rounded once to fp16."""
    T_, H_ = x.shape
    xb = x.reshape(T_, H_ // BK, BK)
    amax = np.max(np.abs(xb), axis=-1)
    scale = np.maximum(amax, 1e-12) / 448.0
    q = np.clip(xb / scale[..., None], -448.0, 448.0).astype(F8_OCP)
    xq = q.astype(np.float32) * scale[..., None]
    return xq.reshape(T_, H_).astype(np.float16)


def kernel(hidden_states, top_k_index, top_k_weights, gate_up_proj,
           gate_up_proj_scale_inv, down_proj, down_proj_scale_inv,
           _trace=False, _tmpdir=None):
    from concourse import bass_utils

    hs = np.ascontiguousarray(np.asarray(hidden_states, dtype=np.float32))
    tki = np.asarray(top_k_index)
    tkw = np.asarray(top_k_weights, dtype=np.float32)

    # ---- host routing (the "all-to-all dispatch") + act quant ----
    xq16_full = _act_quant_fp16(hs)                       # [T, H] fp16
    toks_per_e = []
    for e in range(E):
        toks_per_e.append(np.nonzero((tki == e).any(axis=1))[0])
    max_count = max(len(t) for t in toks_per_e)
    C = max(P, -(-max_count // P) * P)
    NT = C // P

    if C not in _compiled_cache:
        _compiled_cache[C] = _build(C)
    nc = _compiled_cache[C]

    wprep = _prep_weights(gate_up_proj, gate_up_proj_scale_inv, down_proj,
                          down_proj_scale_inv)
    ident = np.eye(P, dtype=np.float16)

    in_maps = []
    for e in range(E):
        toks = toks_per_e[e]
        xq = np.zeros((C, H), np.float16)
        xq[:len(toks)] = xq16_full[toks]
        # pre-transposed lhsT layout: [NT, 128 k-in-block, KB1*128 tokens]
        xqt = np.ascontiguousarray(
            xq.reshape(NT, P, KB1, BK).transpose(0, 3, 2, 1).reshape(NT, P, H))
        wgu, wd = wprep[e]
        in_maps.append({"xqt": xqt, "wgu16": wgu, "wd16": wd, "ident": ident})

    res = bass_utils.run_bass_kernel_spmd(
        nc, in_maps, core_ids=list(range(NCORES)),
        trace=_trace, tmpdir=_tmpdir,
    )

    # ---- host combine ----
    out = np.zeros((T, H), np.float32)
    for e in range(E):
        toks = toks_per_e[e]
        y = res.results[e]["y"]
        for kk in range(TOPK):
            sel = np.nonzero(tki[:, kk] == e)[0]
            pos = np.searchsorted(toks, sel)
            out[sel] += tkw[sel, kk, None] * y[pos]
    if _trace:
        kernel._last_results = res
    return out


# revision 42
# speedup vs baseline: 1.0134x; 1.0134x over previous
"""Trainium2 Bass kernel for nn_FP8Experts (MoE with FP8 block-quantized experts).

Strategy (expert-parallel over 8 NeuronCores):
  - Host: route tokens to experts by top_k_index (each expert's token list,
    padded to a common capacity C); fully dequantize the fp8 block-quantized
    weights to fp16 (w = q * block_scale); apply the reference's dynamic
    per-token/per-128-block fp8 act-quant round-trip to the routed
    activations (bit-exact reference semantics, rounded once to fp16); lay
    both out contraction-major + chunk-major so every DMA is one fat
    contiguous per-partition run.
  - Device (per core = one expert): fp16 weights resident in SBUF, fp16
    matmuls (gate_up -> silu*up -> act-quant of the intermediate -> down)
    accumulated in PSUM fp32. The intermediate act-quant (per-token,
    per-128-block fp8 round-trip matching the reference on a /2-scaled grid:
    224 = 448/2, exact vs OCP e4m3fn away from the denormal floor) runs on
    the vector/scalar engines, overlapped with the matmul stream; its
    transposes (contraction-major for the PE) run on the tensor engine, each
    delayed one chunk-slot so the quant chain's latency stays hidden.
    The first two token tiles are processed chunk-interleaved so the PE
    trails the weight-DMA wavefront without stalling.
  - Host: weighted combine with top_k_weights.
"""

import numpy as np
import ml_dtypes

E, H, I = 8, 2048, 1408
T, TOPK = 4096, 2
BN = BK = 128
NCORES = 8
P = 128
HALF_MAX = 224.0

F8_OCP = ml_dtypes.float8_e4m3fn   # reference grid (max 448)

# gate/up column chunks in matmul-consumption order (g0,u0,g1,u1,g2,u2):
# (orig column offset in [0, 2816), width)
GU_CH = [(0, 512), (1408, 512), (512, 512), (1920, 512),
         (1024, 384), (2432, 384)]
GU_BASE = np.cumsum([0] + [cw for _, cw in GU_CH]).tolist()  # flat offsets
GU_TOT = GU_BASE[-1]            # 2816
KB1 = H // BK                   # 16 contraction blocks for gate_up
KB2 = I // BK                   # 11 contraction blocks for down
WD_CW = 512
WD_TOT = KB2 * H                # flat down-weight cols (chunk-major)

# gate/up paired column chunks: (offset-within-half, width, #inter-blocks)
GCHUNKS = [(0, 512, 4), (512, 512, 4), (1024, 384, 3)]

_compiled_cache = {}
_weights_cache = {}


def _build(C):
    """Build + schedule the per-core Bass kernel for token capacity C."""
    import concourse.bass as bass
    import concourse.mybir as mybir
    import concourse.tile as tile
    from concourse import bacc
    from concourse.masks import make_identity

    f32 = mybir.dt.float32
    f16 = mybir.dt.float16
    f8 = mybir.dt.float8e4
    AF = mybir.ActivationFunctionType
    ALU = mybir.AluOpType
    AX = mybir.AxisListType

    NT = C // P

    nc = bacc.Bacc("TRN2", target_bir_lowering=False, debug=False,
                   num_devices=NCORES)

    xq_d = nc.dram_tensor("xqt", [NT, P, H], f16, kind="ExternalInput").ap()
    wgu_d = nc.dram_tensor("wgu16", [P, GU_TOT * KB1], f16,
                           kind="ExternalInput").ap()
    wd_d = nc.dram_tensor("wd16", [P, WD_TOT], f16, kind="ExternalInput").ap()
    y_d = nc.dram_tensor("y", [C, H], f32, kind="ExternalOutput").ap()

    with tile.TileContext(nc) as tc:
        with (
            tc.tile_pool(name="const", bufs=1) as const,
            tc.tile_pool(name="wpool", bufs=1) as wpool,
            tc.tile_pool(name="qp", bufs=2) as qp,
            tc.tile_pool(name="tp", bufs=2) as tp,
            tc.tile_pool(name="pp", bufs=6, space="PSUM") as pp,
            tc.tile_pool(name="pt", bufs=2, space="PSUM") as pt,
        ):
            # computed on-device: the first DMA pays engine spin-up latency,
            # so sourcing the identity from HBM would delay the first matmul
            ident = const.tile([P, P], f16, name="ident")
            make_identity(nc, ident[:])

            # first use of each engine opcode pays a cold uop-table load;
            # warm every opcode the pipeline uses on tiny tiles first
            wu8 = const.tile([P, 8], f8, name="wu8")
            wu16 = const.tile([P, 8], f16, name="wu16")
            wu32 = const.tile([P, 8], f32, name="wu32")
            nc.vector.tensor_copy(out=wu32[:], in_=ident[:, :8])
            nc.vector.reduce_max(wu32[:, :1], wu32[:, :8], axis=AX.X,
                                 apply_absolute_value=True)
            nc.vector.tensor_scalar_max(wu32[:], wu32[:], 1e-12)
            nc.vector.reciprocal(wu32[:], wu32[:])
            nc.vector.tensor_scalar_mul(wu32[:], wu32[:], 1.0)
            nc.vector.tensor_tensor(out=wu8[:], in0=wu32[:], in1=wu32[:],
                                    op=ALU.mult)
            nc.vector.tensor_tensor(out=wu16[:], in0=wu8[:], in1=wu32[:],
                                    op=ALU.mult)
            nc.scalar.activation(wu16[:], wu16[:], AF.Silu)
            nc.scalar.activation(wu16[:], wu16[:], AF.Copy, scale=1.0)

            # PE warmup: a few dummy matmuls bridge until the first weight
            # quarter-chunk + tile-0 activations land (real matmuls then
            # finish waking the HAM clock-gate).
            ps_warm = pp.tile([P, 512], f32, name="ps", tag="ps")
            for _ in range(52):
                nc.tensor.matmul(ps_warm[:, :P], lhsT=ident[:], rhs=ident[:],
                                 start=True, stop=True)

            # ---------------- resident fp16 weights (host-dequantized) -----
            # chunk-major flat layouts: fat contiguous per-partition DMA runs
            wgu_all = wpool.tile([P, GU_TOT * KB1], f16, name="wgu_all")
            wd_all = wpool.tile([P, WD_TOT], f16, name="wd_all")

            def gu_rhs(ci, kb):
                cw = GU_CH[ci][1]
                b = GU_BASE[ci] * KB1 + kb * cw
                return wgu_all[:, b:b + cw]

            def wd_rhs(hc, kb):
                b = hc * (KB2 * WD_CW) + kb * WD_CW
                return wd_all[:, b:b + WD_CW]

            def load_xq(tt):
                xqT = qp.tile([P, H], f16, name="xqT", tag="xqT")
                nc.sync.dma_start(xqT[:], xq_d[tt])
                return xqT

            # tile 0/1 activation tiles (DMAs issued below, interleaved with
            # the weight stream in first-consumption order)
            hoisted = {0: qp.tile([P, H], f16, name="xqT", tag="xqT")}
            if NT > 1:
                hoisted[1] = qp.tile([P, H], f16, name="xqT", tag="xqT")

            def wdma(lo, hi):
                nc.sync.dma_start(wgu_all[:, lo:hi], wgu_d[:, lo:hi])

            # issue order = order of first consumption by the PE: quarter
            # chunks of the first gate/up pair interleaved with tile-0/1
            # activations, then the remaining chunks in halves
            Q = 4 * 512                                  # kb-quarter cols
            wdma(0, Q)                                   # g0 kb0-3
            nc.sync.dma_start(hoisted[0][:, :H // 2], xq_d[0][:, :H // 2])
            nc.sync.dma_start(hoisted[0][:, H // 2:], xq_d[0][:, H // 2:])
            wdma(Q, 2 * Q)                               # g0 kb4-7
            wdma(2 * Q, 3 * Q)                           # g0 kb8-11
            wdma(3 * Q, 4 * Q)                           # g0 kb12-15
            wdma(4 * Q, 5 * Q)                           # u0 kb0-3
            wdma(5 * Q, 6 * Q)                           # u0 kb4-7
            if NT > 1:
                nc.sync.dma_start(hoisted[1][:], xq_d[1])
            wdma(6 * Q, 7 * Q)                           # u0 kb8-11
            wdma(7 * Q, 8 * Q)                           # u0 kb12-15
            for ci in range(2, len(GU_CH)):
                cw = GU_CH[ci][1]
                b = GU_BASE[ci] * KB1
                half = (KB1 // 2) * cw
                wdma(b, b + half)
                wdma(b + half, b + KB1 * cw)
            for hc in range(4):
                b = hc * (KB2 * WD_CW)
                half = 6 * WD_CW
                nc.sync.dma_start(wd_all[:, b:b + half], wd_d[:, b:b + half])
                nc.sync.dma_start(wd_all[:, b + half:b + KB2 * WD_CW],
                                  wd_d[:, b + half:b + KB2 * WD_CW])

            # ---------------- per-tile emission helpers --------------------
            def tile_begin(tt):
                xqT = hoisted.pop(tt, None)
                if xqT is None:
                    xqT = load_xq(tt)
                if tt + 1 < NT and tt + 1 not in hoisted:
                    hoisted[tt + 1] = load_xq(tt + 1)
                return {
                    "xqT": xqT,
                    "iq16": qp.tile([P, KB2, BK], f16, name="iq16", tag="iq16"),
                    "iqT": qp.tile([P, KB2, BK], f16, name="iqT", tag="iqT"),
                    "amax": qp.tile([P, KB2], f32, name="amax_i", tag="amax_i"),
                    "inv": qp.tile([P, KB2], f32, name="inv_i", tag="inv_i"),
                    "s2": qp.tile([P, KB2], f32, name="s2_i", tag="s2_i"),
                }

            def emit_pair(st, gi):
                """gate+up matmuls for chunk gi, then silu*up + act-quant of
                the resulting intermediate blocks (vector/scalar engines)."""
                off, w, nb = GCHUNKS[gi]
                xqT = st["xqT"]
                ps_g = pp.tile([P, 512], f32, name="ps", tag="ps")[:, :w]
                for kb in range(KB1):
                    nc.tensor.matmul(ps_g, lhsT=xqT[:, kb * BK:(kb + 1) * BK],
                                     rhs=gu_rhs(2 * gi, kb),
                                     start=(kb == 0), stop=(kb == KB1 - 1))
                ps_u = pp.tile([P, 512], f32, name="ps", tag="ps")[:, :w]
                for kb in range(KB1):
                    nc.tensor.matmul(ps_u, lhsT=xqT[:, kb * BK:(kb + 1) * BK],
                                     rhs=gu_rhs(2 * gi + 1, kb),
                                     start=(kb == 0), stop=(kb == KB1 - 1))
                sil = tp.tile([P, 512], f32, name="sil", tag="sil")[:, :w]
                nc.scalar.activation(sil, ps_g, AF.Silu)
                itc = tp.tile([P, 512], f32, name="itc", tag="itc")[:, :w]
                nc.vector.tensor_mul(itc, sil, ps_u)

                b0 = off // BN
                am = st["amax"][:, b0:b0 + nb]
                nc.vector.reduce_max(
                    am, itc.rearrange("p (b k) -> p b k", k=BK),
                    axis=AX.X, apply_absolute_value=True,
                )
                nc.vector.tensor_scalar_max(am, am, 1e-12)
                nc.vector.reciprocal(st["inv"][:, b0:b0 + nb], am)
                nc.vector.tensor_scalar_mul(st["inv"][:, b0:b0 + nb],
                                            st["inv"][:, b0:b0 + nb], HALF_MAX)
                nc.vector.tensor_scalar_mul(st["s2"][:, b0:b0 + nb], am,
                                            1.0 / HALF_MAX)
                qi8 = tp.tile([P, 512], f8, name="qi8", tag="qi8")[:, :w]
                nc.vector.tensor_tensor(
                    out=qi8.rearrange("p (b k) -> p b k", k=BK),
                    in0=itc.rearrange("p (b k) -> p b k", k=BK),
                    in1=st["inv"][:, b0:b0 + nb, None].to_broadcast(
                        [P, nb, BK]),
                    op=ALU.mult,
                )
                # fp8-input DVE ops are slow; split the dequant-to-fp16
                # between DVE and ACT
                nd = nb // 2
                nc.vector.tensor_tensor(
                    out=st["iq16"][:, b0:b0 + nd, :],
                    in0=qi8.rearrange("p (b k) -> p b k", k=BK)[:, :nd],
                    in1=st["s2"][:, b0:b0 + nd, None].to_broadcast(
                        [P, nd, BK]),
                    op=ALU.mult,
                )
                for b in range(nd, nb):
                    nc.scalar.activation(
                        st["iq16"][:, b0 + b, :], qi8[:, b * BK:(b + 1) * BK],
                        AF.Copy, scale=st["s2"][:, b0 + b:b0 + b + 1])

            def emit_T(st, gi):
                """PE transpose of chunk gi's quantized intermediate blocks:
                [token, feat] -> [feat, token]."""
                off, w, nb = GCHUNKS[gi]
                b0 = off // BN
                ps_t = pt.tile([P, 4, P], f16, name="ps_t", tag="ps_t")
                for j in range(nb):
                    nc.tensor.transpose(ps_t[:, j, :],
                                        st["iq16"][:, b0 + j, :], ident[:])
                nc.vector.tensor_copy(out=st["iqT"][:, b0:b0 + nb, :],
                                      in_=ps_t[:, :nb, :])

            def emit_down(st, tt):
                for hc in range(4):
                    ps_y = pp.tile([P, 512], f32, name="ps", tag="ps")
                    for kb in range(KB2):
                        nc.tensor.matmul(ps_y, lhsT=st["iqT"][:, kb, :],
                                         rhs=wd_rhs(hc, kb),
                                         start=(kb == 0), stop=(kb == KB2 - 1))
                    yt = tp.tile([P, 512], f32, name="yt", tag="yt")
                    nc.scalar.copy(yt[:], ps_y[:])
                    nc.sync.dma_start(
                        y_d[tt * P:(tt + 1) * P, hc * 512:(hc + 1) * 512],
                        yt[:])

            # ---------------- main loop over 128-token tiles ----------------
            # Tiles 0/1 run chunk-interleaved: per chunk the PE has ~13.6us
            # of matmuls against ~6us of weight DMA, so it never stalls on
            # the weight stream. Transposes trail by one chunk slot.
            start = 0
            if NT >= 2:
                stA, stB = tile_begin(0), tile_begin(1)
                for gi in range(3):
                    for st in (stA, stB):
                        if gi > 0:
                            emit_T(st, gi - 1)
                        emit_pair(st, gi)
                emit_T(stA, 2)
                emit_down(stA, 0)
                emit_T(stB, 2)
                emit_down(stB, 1)
                start = 2
            for tt in range(start, NT):
                st = tile_begin(tt)
                emit_pair(st, 0)
                emit_pair(st, 1)
                emit_T(st, 0)
                emit_pair(st, 2)
                emit_T(st, 1)
                emit_T(st, 2)
                emit_down(st, tt)

    nc.compile()
    return nc


def _prep_weights(gate_up_proj, gate_up_proj_scale_inv, down_proj,
                  down_proj_scale_inv):
    """Per-expert fully dequantized fp16 weights (w = q * block_scale), in
    chunk-major contraction-major flat layout for fat contiguous DMAs."""
    cached = _weights_cache.get("w")
    if cached is not None and cached[0] is gate_up_proj \
            and cached[1] is down_proj:
        return cached[2]
    NB1, NB2 = 2 * I // BN, H // BN
    out = []
    gup = np.asarray(gate_up_proj)
    gus = np.asarray(gate_up_proj_scale_inv, dtype=np.float32)
    dwn = np.asarray(down_proj)
    dws = np.asarray(down_proj_scale_inv, dtype=np.float32)
    for e in range(E):
        w32 = gup[e].astype(np.float32).reshape(NB1, BN, KB1, BK)
        w32 *= gus[e][:, None, :, None]
        w16T = w32.reshape(2 * I, H).T.astype(np.float16)   # [H, 2I]
        parts = []
        for o, cw in GU_CH:
            blk = w16T[:, o:o + cw].reshape(KB1, P, cw)
            parts.append(blk.transpose(1, 0, 2).reshape(P, KB1 * cw))
        wgu = np.ascontiguousarray(np.concatenate(parts, axis=1))
        w32 = dwn[e].astype(np.float32).reshape(NB2, BN, KB2, BK)
        w32 *= dws[e][:, None, :, None]
        wdT = w32.reshape(H, I).T.astype(np.float16)        # [I, H]
        parts = []
        for hc in range(4):
            blk = wdT[:, hc * WD_CW:(hc + 1) * WD_CW].reshape(KB2, P, WD_CW)
            parts.append(blk.transpose(1, 0, 2).reshape(P, KB2 * WD_CW))
        wd = np.ascontiguousarray(np.concatenate(parts, axis=1))
        out.append((wgu, wd))
    _weights_cache["w"] = (gate_up_proj, down_proj, out)
    return out


def _act_quant_fp16(x):
    """Reference _act_quant_dequant (per-token, per-128-block OCP e4m3fn
    round-trip), rounded once to fp16."""
    T_, H_ = x.shape
    xb = x.reshape(T_, H_ // BK, BK)
    amax = np.max(np.abs(xb), axis=-1)
    scale = np.maximum(amax, 1e-12) / 448.0
    q = np.clip(xb / scale[..., None], -448.0, 448.0).astype(F8_OCP)
    xq = q.astype(np.float32) * scale[..., None]
    return xq.reshape(T_, H_).astype(np.float16)


def kernel(hidden_states, top_k_index, top_k_weights, gate_up_proj,
           gate_up_proj_scale_inv, down_proj, down_proj_scale_inv,
           _trace=False, _tmpdir=None):
    from concourse import bass_utils

    hs = np.ascontiguousarray(np.asarray(hidden_states, dtype=np.float32))
    tki = np.asarray(top_k_index)
    tkw = np.asarray(top_k_weights, dtype=np.float32)

    # ---- host routing (the "all-to-all dispatch") + act quant ----
    xq16_full = _act_quant_fp16(hs)                       # [T, H] fp16
    toks_per_e = []
    for e in range(E):
        toks_per_e.append(np.nonzero((tki == e).any(axis=1))[0])
    max_count = max(len(t) for t in toks_per_e)
    C = max(P, -(-max_count // P) * P)
    NT = C // P

    if C not in _compiled_cache:
        _compiled_cache[C] = _build(C)
    nc = _compiled_cache[C]

    wprep = _prep_weights(gate_up_proj, gate_up_proj_scale_inv, down_proj,
                          down_proj_scale_inv)


    in_maps = []
    for e in range(E):
        toks = toks_per_e[e]
        xq = np.zeros((C, H), np.float16)
        xq[:len(toks)] = xq16_full[toks]
        # pre-transposed lhsT layout: [NT, 128 k-in-block, KB1*128 tokens]
        xqt = np.ascontiguousarray(
            xq.reshape(NT, P, KB1, BK).transpose(0, 3, 2, 1).reshape(NT, P, H))
        wgu, wd = wprep[e]
        in_maps.append({"xqt": xqt, "wgu16": wgu, "wd16": wd})

    res = bass_utils.run_bass_kernel_spmd(
        nc, in_maps, core_ids=list(range(NCORES)),
        trace=_trace, tmpdir=_tmpdir,
    )

    # ---- host combine ----
    out = np.zeros((T, H), np.float32)
    for e in range(E):
        toks = toks_per_e[e]
        y = res.results[e]["y"]
        for kk in range(TOPK):
            sel = np.nonzero(tki[:, kk] == e)[0]
            pos = np.searchsorted(toks, sel)
            out[sel] += tkw[sel, kk, None] * y[pos]
    if _trace:
        kernel._last_results = res
    return out
